# revision 13
# baseline (speedup 1.0000x reference)
"""Trainium2 Bass kernel for nn_MessagePassingLayer (GNN message passing).

Computes, for edges a[i] = (s, t) with edge features e[i] and node
features r:
    out = segment_sum(r[a[:,0]] * e, a[:,1]) + segment_sum(r[a[:,1]] * e, a[:,0])

Strategy (8 NeuronCores, full inputs in / full output out):
  - Expand each edge into its two messages (dst <- r[src] * e[edge]);
    sort messages by destination node on the host and pack consecutive
    destination nodes into "blocks" of <=128 nodes and <=K*128 messages.
    Each core owns a contiguous range of B blocks (a contiguous slice of
    the output rows) -- no cross-core reduction is needed.
  - The shards shipped to each core are r[src] and e[edge] materialized in
    message-slot order as bf16, so the device reads only contiguous streams
    (the host does the permutation indexing; the device does all the math).
  - Device, per group of G blocks: stream the r and e slabs on separate
    HWDGE rings (sync / scalar), multiply them (DVE, bf16), build a one-hot
    selection matrix S[msg, dst_local] from the dst-local ids via
    iota + is_equal (DVE, bf16), and accumulate
        out_block[dst_local, feat] += sum_msg S[msg, dst_local] * prod[msg, feat]
    as K chained 128x128x128 bf16 matmuls into fp32 PSUM -- the segmented
    reduction is a matmul against the one-hot matrix.  Finished blocks are
    copied PSUM->SBUF on the Act engine and written back contiguously, one
    DMA per group.
  - Host maps block-local rows to global node rows (vectorized take).
"""

import numpy as np

import concourse.bass as bass
import concourse.mybir as mybir
import concourse.tile as tile
from concourse.bass_utils import run_bass_kernel_spmd
from concourse.vector_clock import ScopedClock

P = 128
D = 128
N_CORES = 8

BF16 = mybir.dt.bfloat16
NP_BF16 = mybir.dt.np(mybir.dt.bfloat16)

# ---------------------------------------------------------------------------
# Workarounds for the walrus build in this environment, which rejects any
# instruction carrying more than one semaphore wait ("Too many sync wait
# commands").  Tile's tail drain and scheduler can emit such instructions;
# split the extra waits onto dedicated single-wait NoOps.
# ---------------------------------------------------------------------------


def _patched_drain_and_barrier(self, tick_clock, wait_clock):
    nc = self.nc
    carrier = nc.sync.nop(nofuse=True, hint="drain_wait_carrier")
    wait_clock.add_sem_waits(carrier.ins, ScopedClock({None: tick_clock.global_clock}))
    si = carrier.ins.sync_info
    if si is not None and si.on_wait and len(si.on_wait) > 1:
        extras = list(si.on_wait[1:])
        del si.on_wait[1:]
        for w in extras:
            extra = nc.sync.nop(nofuse=True, hint="drain_wait_carrier")
            if extra.ins.sync_info is None:
                extra.ins.sync_info = mybir.SyncInfo(on_wait=[w], on_update=[])
            else:
                extra.ins.sync_info.on_wait.append(w)
    nc.sync.drain()
    nc.all_engine_barrier()
    assert self.sems is not None
    popped = nc._tile_sem_poison_stack.pop()
    assert popped is self._sem_poison
    nc.clear_and_free_semaphores(list(self.sems.allocated().values()))
    nc.all_engine_barrier()


tile.TileContext._drain_and_barrier = _patched_drain_and_barrier


def _split_multi_waits(nc):
    for fn in nc.m.functions:
        for bb in fn.blocks:
            out = []
            for inst in bb.instructions:
                si = inst.sync_info
                if si is not None and si.on_wait is not None and len(si.on_wait) > 1:
                    extras = list(si.on_wait[:-1])
                    del si.on_wait[:-1]
                    for w in extras:
                        out.append(mybir.InstNoOp(
                            text_hint="waitsplit",
                            bass_nofuse=True,
                            name=nc.get_next_instruction_name(),
                            engine=inst.engine,
                            ins=[], outs=[],
                            sync_info=mybir.SyncInfo(on_wait=[w], on_update=[]),
                        ))
                out.append(inst)
            bb.instructions[:] = out


# ---------------------------------------------------------------------------
# Device program
# ---------------------------------------------------------------------------


def build_kernel(B, K, n_cores=N_CORES, gblocks=4, iters=1,
                 dma_only=False, compute_only=False):
    """Per-core inputs:
      rperm  [B, P, K*D] bf16 : r[src] in slot order
      eperm  [B, P, K*D] bf16 : e[edge] in slot order (0 at pad slots)
      dstloc [P, B*K]    bf16 : block-local dst index per slot (-1 pad)
    Output: out [B, P, D] f32 : row (b, p) = accumulated features of the
    p-th node of block b (rows past a block's node count are garbage).
    """
    nc = bass.Bass("TRN2", num_devices=n_cores)
    # p-major layouts: one contiguous run per partition per group-transfer
    # (128 big descriptors instead of 512 small ones per DMA)
    X = K * D
    rperm_t = nc.declare_dram_parameter("rperm", [P, B * X], BF16, isOutput=False)
    eperm_t = nc.declare_dram_parameter("eperm", [P, B * X], BF16, isOutput=False)
    dstloc_t = nc.declare_dram_parameter("dstloc", [P, B * K], BF16, isOutput=False)
    out_t = nc.declare_dram_parameter("out", [P, B * D], BF16, isOutput=True)

    G = gblocks
    groups = [G] * (B // G)
    if B % G:
        groups.append(B % G)

    with tile.TileContext(nc) as tc:
        with (
            tc.tile_pool(name="const", bufs=1) as constp,
            tc.tile_pool(name="idx", bufs=1) as idxp,
            tc.tile_pool(name="rg", bufs=3) as rgp,
            tc.tile_pool(name="eg", bufs=3) as egp,
            tc.tile_pool(name="sm", bufs=3) as smp,
            tc.tile_pool(name="stage", bufs=3) as stagep,
            tc.tile_pool(name="psum", bufs=6, space="PSUM") as psump,
        ):
            # iota value n at column (n, k) of an [n, k]-major [P, P*K] tile,
            # as bf16 (for one-hot building); k innermost keeps every operand
            # of the is_equal packed (stride-1 last dim) for fast DVE modes.
            iota_i = constp.tile([P, P * K], mybir.dt.int32)
            nc.gpsimd.iota(iota_i[:], pattern=[[1, P], [0, K]], base=0,
                           channel_multiplier=0)
            iota_f = constp.tile([P, P * K], BF16)
            nc.vector.tensor_copy(iota_f[:], iota_i[:])

            dstloc_sb = idxp.tile([P, B * K], BF16)
            nc.sync.dma_start(dstloc_sb[:], dstloc_t[:])

            if compute_only:
                rg_c = rgp.tile([P, G * X], BF16)
                eg_c = egp.tile([P, G * X], BF16)
                nc.vector.memset(rg_c[:], 0)
                nc.vector.memset(eg_c[:], 0)

            for _ in range(iters):
                g0 = 0
                for Gc in groups:
                    if compute_only:
                        rg = rg_c
                        eg = eg_c
                        if Gc != G:
                            g0 += Gc
                            continue
                    else:
                        rg = rgp.tile([P, Gc * X], BF16)
                        eg = egp.tile([P, Gc * X], BF16)
                        nc.sync.dma_start(rg[:], rperm_t[:, g0 * X:(g0 + Gc) * X])
                        nc.scalar.dma_start(eg[:], eperm_t[:, g0 * X:(g0 + Gc) * X])
                    if dma_only:
                        stg = stagep.tile([P, Gc * D], BF16)
                        nc.vector.tensor_copy(stg[:], rg[:, :Gc * D])
                        nc.gpsimd.dma_start(
                            out_t[:, g0 * D:(g0 + Gc) * D], stg[:])
                        g0 += Gc
                        continue
                    nc.vector.tensor_mul(rg[:], rg[:], eg[:])
                    stg = stagep.tile([P, Gc * D], BF16)
                    for bl in range(Gc):
                        b = g0 + bl
                        # one-hot S in (n, k)-major layout: S[p, n, k] =
                        # (dstloc[p, k] == n); every operand has a packed
                        # stride-1 16-bit last dim (fast DVE mode).
                        S = smp.tile([P, P * K], BF16)
                        nc.vector.tensor_tensor(
                            out=S[:].rearrange("p (n k) -> p n k", k=K),
                            in0=dstloc_sb[:, b * K:(b + 1) * K]
                                .rearrange("p (o k) -> p o k", o=1)
                                .to_broadcast([P, P, K]),
                            in1=iota_f[:].rearrange("p (n k) -> p n k", k=K),
                            op=mybir.AluOpType.is_equal)
                        ps = psump.tile([P, P], mybir.dt.float32)
                        Sv = S[:].rearrange("p (n k) -> p n k", k=K)
                        for k in range(K):
                            col = (bl * K + k) * P
                            nc.tensor.matmul(
                                ps[:],
                                lhsT=Sv[:, :, k],
                                rhs=rg[:, col:col + P],
                                start=(k == 0), stop=(k == K - 1))
                        nc.scalar.copy(stg[:, bl * D:(bl + 1) * D], ps[:])
                    nc.gpsimd.dma_start(
                        out_t[:, g0 * D:(g0 + Gc) * D], stg[:])
                    g0 += Gc
    _split_multi_waits(nc)
    return nc


# ---------------------------------------------------------------------------
# Host-side sharding / layout
# ---------------------------------------------------------------------------


def preprocess(r, e, a, n_cores=N_CORES):
    """Returns (in_maps, row_maps, B, K) where row_maps[c] = (node_ids, flat
    out-row ids) mapping core c's out buffer rows to global node rows."""
    r = np.asarray(r, dtype=np.float32).astype(NP_BF16)
    e = np.asarray(e, dtype=np.float32).astype(NP_BF16)
    a = np.asarray(a)
    N = r.shape[0]
    E = e.shape[0]
    s = a[:, 0].astype(np.int32)
    t = a[:, 1].astype(np.int32)
    dst = np.concatenate([t, s])
    src = np.concatenate([s, t])
    eid = np.concatenate([np.arange(E, dtype=np.int32)] * 2)

    order = np.argsort(dst, kind="stable").astype(np.int32)
    dst_s = dst[order]
    src_s = src[order]
    eid_s = eid[order]

    deg = np.bincount(dst, minlength=N)
    cum = np.concatenate([[0], np.cumsum(deg)])

    # smallest K whose greedy packing (<=P nodes, <=K*P msgs per block) fits
    K = max(12, int(np.ceil(deg.max() / P)))
    while True:
        cap = K * P
        starts_n = [0]
        starts_m = [0]
        n0 = 0
        ok = True
        while n0 < N:
            m0 = cum[n0]
            n1 = min(np.searchsorted(cum, m0 + cap, side="right") - 1, n0 + P, N)
            if n1 <= n0:
                ok = False
                break
            n0 = n1
            starts_n.append(int(n0))
            starts_m.append(int(cum[n0]))
        if ok:
            break
        K += 1
        if K > 64:
            raise RuntimeError("packing failed")
    nblocks = len(starts_n) - 1
    B = -(-nblocks // n_cores)
    TB = n_cores * B
    cap = K * P
    starts_n += [N] * (TB - nblocks)
    starts_m += [int(cum[N])] * (TB - nblocks)
    starts_n = np.asarray(starts_n, dtype=np.int64)
    starts_m = np.asarray(starts_m, dtype=np.int64)

    # slot (b, p, k) holds sorted-message starts_m[b] + k*P + p
    koff = np.arange(cap, dtype=np.int32).reshape(K, P).T          # [P, K]
    sm = starts_m[:TB].astype(np.int32)[:, None, None] + koff[None, :, :]
    valid = sm < starts_m[1:TB + 1].astype(np.int32)[:, None, None]
    smc = np.where(valid, sm, 0)

    src_slot = src_s[smc]                                          # [TB, P, K]
    eid_slot = eid_s[smc]
    dstloc_all = np.where(
        valid, dst_s[smc] - starts_n[:TB].astype(np.int32)[:, None, None],
        -1).astype(NP_BF16)

    rperm_all = np.empty((TB, P, K, D), dtype=NP_BF16)
    np.take(r, src_slot.reshape(-1), axis=0, out=rperm_all.reshape(-1, D))
    eperm_all = np.empty((TB, P, K, D), dtype=NP_BF16)
    np.take(e, eid_slot.reshape(-1), axis=0, out=eperm_all.reshape(-1, D))
    eperm_all.reshape(-1, D)[~valid.reshape(-1)] = 0.0

    nnode_all = (starts_n[1:TB + 1] - starts_n[:TB]).astype(np.int32)  # [TB]
    pvec = np.arange(P, dtype=np.int32)

    in_maps = []
    row_maps = []
    for c in range(n_cores):
        b0, b1 = c * B, (c + 1) * B
        in_maps.append({
            # p-major: [P, B*K*D] so each device DMA descriptor is one long
            # contiguous run per partition
            "rperm": np.ascontiguousarray(
                rperm_all[b0:b1].transpose(1, 0, 2, 3).reshape(P, B * K * D)),
            "eperm": np.ascontiguousarray(
                eperm_all[b0:b1].transpose(1, 0, 2, 3).reshape(P, B * K * D)),
            "dstloc": np.ascontiguousarray(
                dstloc_all[b0:b1].transpose(1, 0, 2).reshape(P, B * K)),
        })
        # out buffer row (p, b) -> global node starts_n[b0 + b] + p  (p < nnode)
        ok_rows = pvec[None, :] < nnode_all[b0:b1, None]              # [B, P]
        bv, pv = np.nonzero(ok_rows)
        row_maps.append((
            (starts_n[b0:b1][bv] + pv).astype(np.int64),  # global node ids
            bv.astype(np.int64), pv.astype(np.int64),
        ))
    return in_maps, row_maps, B, K


def assemble(results, row_maps, N):
    out = np.empty((N, D), dtype=np.float32)
    for c, (nodes, bv, pv) in enumerate(row_maps):
        # device out is [P, B*D]: row (b, p) lives at [p, b*D:(b+1)*D]
        pb = results[c]["out"].reshape(P, -1, D)
        out[nodes] = pb[pv, bv]
    return out


# ---------------------------------------------------------------------------
# Entry point
# ---------------------------------------------------------------------------


def kernel(r, e, a):
    in_maps, row_maps, B, K = preprocess(r, e, a, N_CORES)
    nc = build_kernel(B, K, N_CORES, gblocks=4, iters=1)
    res = run_bass_kernel_spmd(nc, in_maps, list(range(N_CORES)))
    return assemble(res.results, row_maps, np.asarray(r).shape[0])


# revision 22
# speedup vs baseline: 1.0216x; 1.0216x over previous
"""Trainium2 Bass kernel for nn_MessagePassingLayer (GNN message passing).

Computes, for edges a[i] = (s, t) with edge features e[i] and node
features r:
    out = segment_sum(r[a[:,0]] * e, a[:,1]) + segment_sum(r[a[:,1]] * e, a[:,0])

Strategy (8 NeuronCores, full inputs in / full output out):
  - Expand each edge into its two messages (dst <- r[src] * e[edge]);
    sort messages by destination node on the host and pack consecutive
    destination nodes into "blocks" of <=128 nodes and <=K*128 messages.
    Each core owns a contiguous range of B blocks (a contiguous slice of
    the output rows) -- no cross-core reduction is needed.
  - The shards shipped to each core are r[src] and e[edge] materialized in
    message-slot order as bf16 in partition-major layout (one long
    contiguous run per partition per transfer), so the device reads only
    contiguous streams (the host does the permutation indexing; the device
    does all the math).
  - Device, per group of G blocks: stream the r and e slabs on separate
    DMA rings, multiply them (DVE, bf16), build a one-hot selection matrix
    S[msg, dst_local] from the dst-local ids via iota + is_equal (DVE,
    bf16, all operands packed 16-bit), and accumulate
        out_block[dst_local, feat] += sum_msg S[msg, dst_local] * prod[msg, feat]
    as K chained 128x128x128 bf16 matmuls into fp32 PSUM -- the segmented
    reduction is a matmul against the one-hot matrix.  Finished blocks are
    copied PSUM->SBUF (bf16) on the Act engine and written back
    contiguously, one DMA per group.
  - Host maps block-local rows to global node rows (vectorized take).

Measured on trn2 (8 cores, within-run A/B): the kernel is DMA-bound --
streams-only timing ~160us vs full kernel ~165us at ~81MB/core (~507GB/s
effective); fp32 baseline was PE-bound (fp32 matmul = 4 cycles/row).
Output max rel err vs the f32 reference ~5e-3 (gate 2e-2).
"""

import numpy as np

import concourse.bass as bass
import concourse.mybir as mybir
import concourse.tile as tile
from concourse.bass_utils import run_bass_kernel_spmd
from concourse.vector_clock import ScopedClock

P = 128
D = 128
N_CORES = 8

BF16 = mybir.dt.bfloat16
NP_BF16 = mybir.dt.np(mybir.dt.bfloat16)

# ---------------------------------------------------------------------------
# Workarounds for the walrus build in this environment, which rejects any
# instruction carrying more than one semaphore wait ("Too many sync wait
# commands").  Tile's tail drain and scheduler can emit such instructions;
# split the extra waits onto dedicated single-wait NoOps.
# ---------------------------------------------------------------------------


def _patched_drain_and_barrier(self, tick_clock, wait_clock):
    nc = self.nc
    carrier = nc.sync.nop(nofuse=True, hint="drain_wait_carrier")
    wait_clock.add_sem_waits(carrier.ins, ScopedClock({None: tick_clock.global_clock}))
    si = carrier.ins.sync_info
    if si is not None and si.on_wait and len(si.on_wait) > 1:
        extras = list(si.on_wait[1:])
        del si.on_wait[1:]
        for w in extras:
            extra = nc.sync.nop(nofuse=True, hint="drain_wait_carrier")
            if extra.ins.sync_info is None:
                extra.ins.sync_info = mybir.SyncInfo(on_wait=[w], on_update=[])
            else:
                extra.ins.sync_info.on_wait.append(w)
    nc.sync.drain()
    nc.all_engine_barrier()
    assert self.sems is not None
    popped = nc._tile_sem_poison_stack.pop()
    assert popped is self._sem_poison
    nc.clear_and_free_semaphores(list(self.sems.allocated().values()))
    nc.all_engine_barrier()


tile.TileContext._drain_and_barrier = _patched_drain_and_barrier


def _split_multi_waits(nc):
    for fn in nc.m.functions:
        for bb in fn.blocks:
            out = []
            for inst in bb.instructions:
                si = inst.sync_info
                if si is not None and si.on_wait is not None and len(si.on_wait) > 1:
                    extras = list(si.on_wait[:-1])
                    del si.on_wait[:-1]
                    for w in extras:
                        out.append(mybir.InstNoOp(
                            text_hint="waitsplit",
                            bass_nofuse=True,
                            name=nc.get_next_instruction_name(),
                            engine=inst.engine,
                            ins=[], outs=[],
                            sync_info=mybir.SyncInfo(on_wait=[w], on_update=[]),
                        ))
                out.append(inst)
            bb.instructions[:] = out


# ---------------------------------------------------------------------------
# Device program
# ---------------------------------------------------------------------------


def build_kernel(B, K, n_cores=N_CORES, gblocks=4, iters=1,
                 dma_only=False, compute_only=False, s_layout="nk",
                 ring_mode="2q"):
    """Per-core inputs:
      rperm  [B, P, K*D] bf16 : r[src] in slot order
      eperm  [B, P, K*D] bf16 : e[edge] in slot order (0 at pad slots)
      dstloc [P, B*K]    bf16 : block-local dst index per slot (-1 pad)
    Output: out [B, P, D] f32 : row (b, p) = accumulated features of the
    p-th node of block b (rows past a block's node count are garbage).
    """
    nc = bass.Bass("TRN2", num_devices=n_cores)
    # p-major layouts: one contiguous run per partition per group-transfer
    # (128 big descriptors instead of 512 small ones per DMA)
    X = K * D
    rperm_t = nc.declare_dram_parameter("rperm", [P, B * X], BF16, isOutput=False)
    eperm_t = nc.declare_dram_parameter("eperm", [P, B * X], BF16, isOutput=False)
    dstloc_t = nc.declare_dram_parameter("dstloc", [P, B * K], BF16, isOutput=False)
    out_t = nc.declare_dram_parameter("out", [P, B * D], BF16, isOutput=True)

    G = gblocks
    groups = [G] * (B // G)
    if B % G:
        groups.append(B % G)

    with tile.TileContext(nc) as tc:
        with (
            tc.tile_pool(name="const", bufs=1) as constp,
            tc.tile_pool(name="idx", bufs=1) as idxp,
            tc.tile_pool(name="rg", bufs=3) as rgp,
            tc.tile_pool(name="eg", bufs=3) as egp,
            tc.tile_pool(name="sm", bufs=3) as smp,
            tc.tile_pool(name="stage", bufs=3) as stagep,
            tc.tile_pool(name="psum", bufs=6, space="PSUM") as psump,
        ):
            # iota value n at column (n, k) [or (k, n)] of a [P, P*K] tile,
            # as bf16 (for one-hot building); k innermost keeps every operand
            # of the is_equal packed (stride-1 last dim) for fast DVE modes.
            iota_i = constp.tile([P, P * K], mybir.dt.int32)
            pat = [[1, P], [0, K]] if s_layout == "nk" else [[0, K], [1, P]]
            nc.gpsimd.iota(iota_i[:], pattern=pat, base=0,
                           channel_multiplier=0)
            iota_f = constp.tile([P, P * K], BF16)
            nc.vector.tensor_copy(iota_f[:], iota_i[:])

            dstloc_sb = idxp.tile([P, B * K], BF16)
            nc.sync.dma_start(dstloc_sb[:], dstloc_t[:])

            if compute_only:
                rg_c = rgp.tile([P, G * X], BF16)
                eg_c = egp.tile([P, G * X], BF16)
                nc.vector.memset(rg_c[:], 0)
                nc.vector.memset(eg_c[:], 0)

            for _ in range(iters):
                g0 = 0
                for Gc in groups:
                    if compute_only:
                        rg = rg_c
                        eg = eg_c
                        if Gc != G:
                            g0 += Gc
                            continue
                    else:
                        rg = rgp.tile([P, Gc * X], BF16)
                        eg = egp.tile([P, Gc * X], BF16)
                        c0, c1 = g0 * X, (g0 + Gc) * X
                        if ring_mode == "3q":
                            # balance ~28MB per ring: SP and Act take ~69% of
                            # one stream each, the Pool SWDGE ring the tails
                            h = (Gc * X * 11 // 16) // D * D
                            nc.sync.dma_start(rg[:, :h], rperm_t[:, c0:c0 + h])
                            nc.gpsimd.dma_start(rg[:, h:], rperm_t[:, c0 + h:c1])
                            nc.scalar.dma_start(eg[:, :h], eperm_t[:, c0:c0 + h])
                            nc.gpsimd.dma_start(eg[:, h:], eperm_t[:, c0 + h:c1])
                        else:
                            nc.sync.dma_start(rg[:], rperm_t[:, c0:c1])
                            nc.scalar.dma_start(eg[:], eperm_t[:, c0:c1])
                    if dma_only:
                        stg = stagep.tile([P, Gc * D], BF16)
                        nc.vector.tensor_copy(stg[:], rg[:, :Gc * D])
                        nc.gpsimd.dma_start(
                            out_t[:, g0 * D:(g0 + Gc) * D], stg[:])
                        g0 += Gc
                        continue
                    nc.vector.tensor_mul(rg[:], rg[:], eg[:])
                    stg = stagep.tile([P, Gc * D], BF16)
                    for bl in range(Gc):
                        b = g0 + bl
                        # one-hot S: S[p, n, k] = (dstloc[p, k] == n).
                        # "nk" layout: packed stride-1 16-bit last dims on
                        # every is_equal operand (fast DVE mode) but a
                        # stride-K lhsT for the matmuls; "kn": broadcast in0
                        # (slower DVE path) but contiguous lhsT.
                        S = smp.tile([P, P * K], BF16)
                        if s_layout == "nk":
                            nc.vector.tensor_tensor(
                                out=S[:].rearrange("p (n k) -> p n k", k=K),
                                in0=dstloc_sb[:, b * K:(b + 1) * K]
                                    .rearrange("p (o k) -> p o k", o=1)
                                    .to_broadcast([P, P, K]),
                                in1=iota_f[:].rearrange("p (n k) -> p n k", k=K),
                                op=mybir.AluOpType.is_equal)
                            Sv = S[:].rearrange("p (n k) -> p n k", k=K)
                            lhs = [Sv[:, :, k] for k in range(K)]
                        else:
                            nc.vector.tensor_tensor(
                                out=S[:].rearrange("p (k n) -> p k n", n=P),
                                in0=dstloc_sb[:, b * K:(b + 1) * K]
                                    .to_broadcast([P, K, P]),
                                in1=iota_f[:].rearrange("p (k n) -> p k n", n=P),
                                op=mybir.AluOpType.is_equal)
                            lhs = [S[:, k * P:(k + 1) * P] for k in range(K)]
                        ps = psump.tile([P, P], mybir.dt.float32)
                        for k in range(K):
                            col = (bl * K + k) * P
                            nc.tensor.matmul(
                                ps[:],
                                lhsT=lhs[k],
                                rhs=rg[:, col:col + P],
                                start=(k == 0), stop=(k == K - 1))
                        nc.scalar.copy(stg[:, bl * D:(bl + 1) * D], ps[:])
                    nc.gpsimd.dma_start(
                        out_t[:, g0 * D:(g0 + Gc) * D], stg[:])
                    g0 += Gc
    _split_multi_waits(nc)
    return nc


# ---------------------------------------------------------------------------
# Host-side sharding / layout
# ---------------------------------------------------------------------------


def preprocess(r, e, a, n_cores=N_CORES):
    """Returns (in_maps, row_maps, B, K) where row_maps[c] = (node_ids, flat
    out-row ids) mapping core c's out buffer rows to global node rows."""
    r = np.asarray(r, dtype=np.float32).astype(NP_BF16)
    e = np.asarray(e, dtype=np.float32).astype(NP_BF16)
    a = np.asarray(a)
    N = r.shape[0]
    E = e.shape[0]
    s = a[:, 0].astype(np.int32)
    t = a[:, 1].astype(np.int32)
    dst = np.concatenate([t, s])
    src = np.concatenate([s, t])
    eid = np.concatenate([np.arange(E, dtype=np.int32)] * 2)

    order = np.argsort(dst, kind="stable").astype(np.int32)
    dst_s = dst[order]
    src_s = src[order]
    eid_s = eid[order]

    deg = np.bincount(dst, minlength=N)
    cum = np.concatenate([[0], np.cumsum(deg)])

    # smallest K whose greedy packing (<=P nodes, <=K*P msgs per block) fits
    K = max(12, int(np.ceil(deg.max() / P)))
    while True:
        cap = K * P
        starts_n = [0]
        starts_m = [0]
        n0 = 0
        ok = True
        while n0 < N:
            m0 = cum[n0]
            n1 = min(np.searchsorted(cum, m0 + cap, side="right") - 1, n0 + P, N)
            if n1 <= n0:
                ok = False
                break
            n0 = n1
            starts_n.append(int(n0))
            starts_m.append(int(cum[n0]))
        if ok:
            break
        K += 1
        if K > 64:
            raise RuntimeError("packing failed")
    nblocks = len(starts_n) - 1
    B = -(-nblocks // n_cores)
    TB = n_cores * B
    cap = K * P
    starts_n += [N] * (TB - nblocks)
    starts_m += [int(cum[N])] * (TB - nblocks)
    starts_n = np.asarray(starts_n, dtype=np.int64)
    starts_m = np.asarray(starts_m, dtype=np.int64)

    # slot (b, p, k) holds sorted-message starts_m[b] + k*P + p
    koff = np.arange(cap, dtype=np.int32).reshape(K, P).T          # [P, K]
    sm = starts_m[:TB].astype(np.int32)[:, None, None] + koff[None, :, :]
    valid = sm < starts_m[1:TB + 1].astype(np.int32)[:, None, None]
    smc = np.where(valid, sm, 0)

    src_slot = src_s[smc]                                          # [TB, P, K]
    eid_slot = eid_s[smc]
    dstloc_all = np.where(
        valid, dst_s[smc] - starts_n[:TB].astype(np.int32)[:, None, None],
        -1).astype(NP_BF16)

    rperm_all = np.empty((TB, P, K, D), dtype=NP_BF16)
    np.take(r, src_slot.reshape(-1), axis=0, out=rperm_all.reshape(-1, D))
    eperm_all = np.empty((TB, P, K, D), dtype=NP_BF16)
    np.take(e, eid_slot.reshape(-1), axis=0, out=eperm_all.reshape(-1, D))
    eperm_all.reshape(-1, D)[~valid.reshape(-1)] = 0.0

    nnode_all = (starts_n[1:TB + 1] - starts_n[:TB]).astype(np.int32)  # [TB]
    pvec = np.arange(P, dtype=np.int32)

    in_maps = []
    row_maps = []
    for c in range(n_cores):
        b0, b1 = c * B, (c + 1) * B
        in_maps.append({
            # p-major: [P, B*K*D] so each device DMA descriptor is one long
            # contiguous run per partition
            "rperm": np.ascontiguousarray(
                rperm_all[b0:b1].transpose(1, 0, 2, 3).reshape(P, B * K * D)),
            "eperm": np.ascontiguousarray(
                eperm_all[b0:b1].transpose(1, 0, 2, 3).reshape(P, B * K * D)),
            "dstloc": np.ascontiguousarray(
                dstloc_all[b0:b1].transpose(1, 0, 2).reshape(P, B * K)),
        })
        # out buffer row (p, b) -> global node starts_n[b0 + b] + p  (p < nnode)
        ok_rows = pvec[None, :] < nnode_all[b0:b1, None]              # [B, P]
        bv, pv = np.nonzero(ok_rows)
        row_maps.append((
            (starts_n[b0:b1][bv] + pv).astype(np.int64),  # global node ids
            bv.astype(np.int64), pv.astype(np.int64),
        ))
    return in_maps, row_maps, B, K


def assemble(results, row_maps, N):
    out = np.empty((N, D), dtype=np.float32)
    for c, (nodes, bv, pv) in enumerate(row_maps):
        # device out is [P, B*D]: row (b, p) lives at [p, b*D:(b+1)*D]
        pb = results[c]["out"].reshape(P, -1, D)
        out[nodes] = pb[pv, bv]
    return out


# ---------------------------------------------------------------------------
# Entry point
# ---------------------------------------------------------------------------


def kernel(r, e, a):
    in_maps, row_maps, B, K = preprocess(r, e, a, N_CORES)
    nc = build_kernel(B, K, N_CORES, gblocks=4, iters=1)
    res = run_bass_kernel_spmd(nc, in_maps, list(range(N_CORES)))
    return assemble(res.results, row_maps, np.asarray(r).shape[0])
